# revision 13
# baseline (speedup 1.0000x reference)
"""MoE NaiveGate kernel for Trainium2 (8 NeuronCores, data-parallel over tokens).

Computes, for inp [16384, 2048] f32, W [64, 2048] f32, b [64] f32:
    gate = inp @ W.T + b            # [16384, 64]
    top_val, top_idx = top_k(gate, 2)
    gate_score = softmax(top_val)[:, None, :]   # [16384, 1, 2]
    returns (top_idx.reshape(-1) int32, gate_score f32)

Sharding: token dim split 8 ways (2048 tokens/core); W, b replicated.

Per-core pipeline (all within one NeuronCore, Tile-scheduled):
  - DMA 4MB natural tiles [128 tok, 4, 2048 d] (fully contiguous rows).
  - PE transposes 128x128 blocks (inp must be d-major for the matmul
    contraction; fp32 DMA transpose does not exist on trn2), two blocks'
    worth per 2-bank PSUM tile; drains split between DVE and ACT.
  - fp32 matmuls with W.T stationary, row-packed as concurrent K=64 pairs
    (tile_position rows 0/64): gate.T [64, 512] accumulated in PSUM.
  - PE transpose of gate.T back to [tok, 64], bias added during PSUM drain.
  - HW MAX8/MAX_INDEX top-k, batched 2-way softmax, two small output DMAs.
"""

import sys

for _p in ("/opt/trn_rl_repo",):
    if _p not in sys.path:
        sys.path.insert(0, _p)

import numpy as np

import concourse.bass as bass
import concourse.bacc as bacc
import concourse.mybir as mybir
import concourse.tile as tile
from concourse.masks import make_identity

N_CORES = 8
TOKENS = 16384
D = 2048
E = 64
TOP_K = 2

T_CORE = TOKENS // N_CORES  # 2048 tokens per core
GROUP = 512                 # tokens per matmul moving operand
N_GROUPS = T_CORE // GROUP  # 4
SUBT = GROUP // 128         # 4 token-subtiles per group
KCH = D // 128              # 16 contraction chunks
N_TILES = T_CORE // 128     # 16 token tiles per core

f32 = mybir.dt.float32
f32r = mybir.dt.float32r
f16 = mybir.dt.float16
u32 = mybir.dt.uint32


def build_gate_kernel(matmul_mode: str = "fp16x3") -> bass.Bass:
    """matmul_mode:
    'f32row' - exact fp32, each 128-chunk contraction issued as two
               concurrent K=64 row-packed matmuls (rows 0-63 / 64-127)
               accumulating into the same [64, 512] PSUM tile.
    'f32'    - exact fp32, plain K=128 matmuls (4 cyc/row).
    'f32r'   - fp32 data through the fast reduced-precision PE path
               (~5e-4 logit error; can flip near-tied expert pairs).
    """
    nc = bacc.Bacc(None)

    inp = nc.declare_dram_parameter("inp", [T_CORE, D], f32, isOutput=False)
    W = nc.declare_dram_parameter("W", [E, D], f32, isOutput=False)
    b = nc.declare_dram_parameter("b", [E], f32, isOutput=False)
    out_idx = nc.declare_dram_parameter("out_idx", [T_CORE, TOP_K], u32, isOutput=True)
    out_score = nc.declare_dram_parameter("out_score", [T_CORE, TOP_K], f32, isOutput=True)

    def mm_cast(ap):
        return ap.bitcast(f32r) if matmul_mode == "f32r" else ap

    with tile.TileContext(nc) as tc:
        with (
            tc.tile_pool(name="const", bufs=1) as const_pool,
            tc.tile_pool(name="nat", bufs=2) as nat_pool,
            tc.tile_pool(name="inpT", bufs=2) as inpT_pool,
            tc.tile_pool(name="inpTh", bufs=2) as inpTh_pool,
            tc.tile_pool(name="inpTl", bufs=2) as inpTl_pool,
            tc.tile_pool(name="small", bufs=4) as small_pool,
            tc.tile_pool(name="acc", bufs=1) as acc_pool,
            tc.tile_pool(name="ps_t", bufs=2, space="PSUM") as ps_t_pool,
            tc.tile_pool(name="ps_g", bufs=2, space="PSUM") as ps_g_pool,
            tc.tile_pool(name="ps_s", bufs=2, space="PSUM") as ps_s_pool,
        ):
            # ---- input stream first so the big DMAs hit the queues early ----
            nats = []
            for g in range(N_GROUPS):
                nat = nat_pool.tile([128, SUBT, D], f32)
                nc.sync.dma_start(
                    out=nat[:],
                    in_=inp[g * GROUP:(g + 1) * GROUP, :].rearrange(
                        "(s p) d -> p s d", p=128
                    ),
                )
                nats.append(nat)

            # ---- constants / prologue ----
            ident = const_pool.tile([128, 128], f32)
            make_identity(nc, ident)

            # bias replicated to all 128 partitions during DMA
            bias_sb = const_pool.tile([128, E], f32)
            b_ap = b[:]
            bias_bcast = bass.AP(
                tensor=b_ap.tensor, offset=b_ap.offset, ap=[[0, 128]] + list(b_ap.ap)
            )
            nc.gpsimd.dma_start(out=bias_sb[:], in_=bias_bcast)

            # W [64, 2048] -> WT_sb [128, kch, 64] (d-major). In f32r mode the
            # drain copies write through an f32r-bitcast AP: walrus requires
            # f32r matmul inputs to be rounded by the producer.
            w_sb = const_pool.tile([E, D], f32)
            nc.scalar.dma_start(out=w_sb[:], in_=W[:, :])
            WT_sb = const_pool.tile([128, KCH, E], f32)
            WTh = const_pool.tile([128, KCH, E], f16)
            WTl = const_pool.tile([128, KCH, E], f16)
            for k in range(KCH):
                pw = ps_s_pool.tile([128, E], f32, tag="ps_small")
                nc.tensor.transpose(pw[:], w_sb[:, k * 128:(k + 1) * 128], ident[:E, :E])
                nc.vector.tensor_copy(mm_cast(WT_sb[:, k, :]), pw[:])
                if matmul_mode == "fp16x3":
                    nc.vector.tensor_copy(WTh[:, k, :], pw[:])
                    nc.vector.tensor_sub(WTl[:, k, :], pw[:], WTh[:, k, :])

            # per-core collected top-8 values / indices
            vals8 = acc_pool.tile([128, N_TILES, 8], f32)
            idx8 = acc_pool.tile([128, N_TILES, 8], u32)

            pending = None  # deferred gate epilogue of the previous group

            def gate_epilogue(g, gate_ps):
                # gate.T [64, 512] psum -> sbuf, then 4 PE transposes back to
                # [128 tok, 64] and per-tile top-k.
                gsbT = small_pool.tile([E, GROUP], f32, tag="gsbT")
                nc.vector.tensor_copy(gsbT[:], gate_ps[:])
                for s in range(SUBT):
                    ti = g * SUBT + s
                    pt = ps_s_pool.tile([128, E], f32, tag="ps_small")
                    nc.tensor.transpose(
                        pt[:], gsbT[:, s * 128:(s + 1) * 128], ident[:E, :E]
                    )
                    gate_sb = small_pool.tile([128, E], f32, tag="gate_sb")
                    nc.vector.tensor_add(gate_sb[:], pt[:], bias_sb[:])
                    nc.vector.max(out=vals8[:, ti], in_=gate_sb[:])
                    nc.vector.max_index(
                        out=idx8[:, ti], in_max=vals8[:, ti], in_values=gate_sb[:]
                    )

            # ---- main loop over 512-token groups ----
            for g in range(N_GROUPS):
                nat = nats[g]
                if matmul_mode == "fp16x3":
                    inpTh = inpTh_pool.tile([128, KCH, GROUP], f16)
                    inpTl = inpTl_pool.tile([128, KCH, GROUP], f16)
                else:
                    inpT = inpT_pool.tile([128, KCH, GROUP], f32)
                # transpose 2 k-chunks into one 2-bank psum tile, then one
                # bigger drain; drains alternate DVE / ACT (2:1)
                for kk in range(KCH // 2):
                    pt = ps_t_pool.tile([128, 2, GROUP], f32)
                    for half in range(2):
                        k = 2 * kk + half
                        for s in range(SUBT):
                            nc.tensor.matmul(
                                pt[:, half, s * 128:(s + 1) * 128],
                                nat[:, s, k * 128:(k + 1) * 128],
                                ident[:],
                                is_transpose=True,
                                start=(s == 0),
                                stop=(s == SUBT - 1),
                            )
                    if matmul_mode == "fp16x3":
                        dh = inpTh[:, 2 * kk:2 * kk + 2, :]
                        dl = inpTl[:, 2 * kk:2 * kk + 2, :]
                        nc.vector.tensor_copy(dh, pt[:])
                        nc.vector.tensor_sub(dl, pt[:], dh)
                    else:
                        dst = inpT[:, 2 * kk:2 * kk + 2, :]
                        nc.vector.tensor_copy(mm_cast(dst), pt[:])

                if pending is not None:
                    gate_epilogue(*pending)
                    pending = None

                gate_ps = ps_g_pool.tile([E, GROUP], f32, tag="gate_ps")
                if matmul_mode == "fp16x3":
                    passes = [(WTh, inpTh), (WTh, inpTl), (WTl, inpTh)]
                    n_mm = KCH * len(passes)
                    i_mm = 0
                    for k in range(KCH):
                        for (wt, xt) in passes:
                            nc.tensor.matmul(
                                gate_ps[:],
                                wt[:, k, :],
                                xt[:, k, :],
                                start=(i_mm == 0),
                                stop=(i_mm == n_mm - 1),
                            )
                            i_mm += 1
                elif matmul_mode == "f32row":
                    # concurrent K=64 row-packed pairs, one accumulation group
                    for k in range(KCH):
                        for h in range(2):
                            nc.tensor.matmul(
                                gate_ps[:],
                                WT_sb[h * E:(h + 1) * E, k, :],
                                inpT[h * E:(h + 1) * E, k, :],
                                start=(k == 0 and h == 0),
                                stop=(k == KCH - 1 and h == 1),
                                skip_group_check=True,
                            )
                else:
                    for k in range(KCH):
                        nc.tensor.matmul(
                            gate_ps[:],
                            mm_cast(WT_sb[:, k, :]),
                            mm_cast(inpT[:, k, :]),
                            start=(k == 0),
                            stop=(k == KCH - 1),
                        )
                pending = (g, gate_ps)

            gate_epilogue(*pending)

            # ---- batched softmax over the two top logits ----
            v0 = vals8[:, :, 0]
            v1 = vals8[:, :, 1]
            d_t = small_pool.tile([128, N_TILES], f32, tag="soft")
            e_t = small_pool.tile([128, N_TILES], f32, tag="soft")
            den_t = small_pool.tile([128, N_TILES], f32, tag="soft")
            score_sb = acc_pool.tile([128, N_TILES, TOP_K], f32)
            nc.vector.tensor_sub(d_t[:], v1, v0)
            nc.scalar.activation(e_t[:], d_t[:], mybir.ActivationFunctionType.Exp)
            nc.vector.tensor_scalar_add(den_t[:], e_t[:], 1.0)
            nc.vector.reciprocal(score_sb[:, :, 0], den_t[:])
            nc.vector.tensor_mul(score_sb[:, :, 1], e_t[:], score_sb[:, :, 0])

            # ---- outputs ----
            nc.sync.dma_start(
                out=out_idx.rearrange("(tl p) k -> p tl k", p=128),
                in_=idx8[:, :, 0:TOP_K],
            )
            nc.sync.dma_start(
                out=out_score.rearrange("(tl p) k -> p tl k", p=128),
                in_=score_sb[:],
            )

    nc.finalize()
    return nc


_NC_CACHE: dict = {}


def _get_nc(matmul_mode: str) -> bass.Bass:
    if matmul_mode not in _NC_CACHE:
        _NC_CACHE[matmul_mode] = build_gate_kernel(matmul_mode)
    return _NC_CACHE[matmul_mode]


def kernel(inp, W, b, matmul_mode: str = "fp16x3", trace: bool = False, **run_kwargs):
    from concourse.bass_utils import run_bass_kernel_spmd

    inp = np.ascontiguousarray(np.asarray(inp, dtype=np.float32))
    W = np.ascontiguousarray(np.asarray(W, dtype=np.float32))
    b = np.ascontiguousarray(np.asarray(b, dtype=np.float32))
    assert inp.shape == (TOKENS, D) and W.shape == (E, D) and b.shape == (E,)

    nc = _get_nc(matmul_mode)
    in_maps = [
        {"inp": inp[c * T_CORE:(c + 1) * T_CORE], "W": W, "b": b}
        for c in range(N_CORES)
    ]
    res = run_bass_kernel_spmd(
        nc, in_maps, core_ids=list(range(N_CORES)), trace=trace, **run_kwargs
    )
    kernel.last_result = res

    idx = np.concatenate([res.results[c]["out_idx"] for c in range(N_CORES)], axis=0)
    score = np.concatenate(
        [res.results[c]["out_score"] for c in range(N_CORES)], axis=0
    )
    gate_top_k_idx = idx.astype(np.int32).reshape(-1)
    gate_score = score.reshape(TOKENS, 1, TOP_K).astype(np.float32)
    return (gate_top_k_idx, gate_score)


# revision 17
# speedup vs baseline: 1.2213x; 1.2213x over previous
"""MoE NaiveGate kernel for Trainium2 (8 NeuronCores, data-parallel over tokens).

Computes, for inp [16384, 2048] f32, W [64, 2048] f32, b [64] f32:
    gate = inp @ W.T + b            # [16384, 64]
    top_val, top_idx = top_k(gate, 2)
    gate_score = softmax(top_val)[:, None, :]   # [16384, 1, 2]
    returns (top_idx.reshape(-1) int32, gate_score f32)

Sharding: token dim split 8 ways (2048 tokens/core); W, b replicated.

Per-core pipeline (all within one NeuronCore, Tile-scheduled):
  - DMA 4MB natural tiles [128 tok, 4, 2048 d] (fully contiguous rows).
  - PE transposes 128x128 blocks (inp must be d-major for the matmul
    contraction; fp32 DMA transpose does not exist on trn2), two blocks'
    worth per 2-bank PSUM tile; drains split between DVE and ACT.
  - fp32 matmuls with W.T stationary, row-packed as concurrent K=64 pairs
    (tile_position rows 0/64): gate.T [64, 512] accumulated in PSUM.
  - PE transpose of gate.T back to [tok, 64], bias added during PSUM drain.
  - HW MAX8/MAX_INDEX top-k, batched 2-way softmax, two small output DMAs.
"""

import sys

for _p in ("/opt/trn_rl_repo",):
    if _p not in sys.path:
        sys.path.insert(0, _p)

import numpy as np

import concourse.bass as bass
import concourse.bacc as bacc
import concourse.mybir as mybir
import concourse.tile as tile
from concourse.masks import make_identity

N_CORES = 8
TOKENS = 16384
D = 2048
E = 64
TOP_K = 2

T_CORE = TOKENS // N_CORES  # 2048 tokens per core
GROUP = 512                 # tokens per matmul moving operand
N_GROUPS = T_CORE // GROUP  # 4
SUBT = GROUP // 128         # 4 token-subtiles per group
KCH = D // 128              # 16 contraction chunks
N_TILES = T_CORE // 128     # 16 token tiles per core

f32 = mybir.dt.float32
f32r = mybir.dt.float32r
f16 = mybir.dt.float16
u32 = mybir.dt.uint32


def build_gate_kernel(matmul_mode: str = "fp16x3") -> bass.Bass:
    """matmul_mode:
    'f32row' - exact fp32, each 128-chunk contraction issued as two
               concurrent K=64 row-packed matmuls (rows 0-63 / 64-127)
               accumulating into the same [64, 512] PSUM tile.
    'f32'    - exact fp32, plain K=128 matmuls (4 cyc/row).
    'f32r'   - fp32 data through the fast reduced-precision PE path
               (~5e-4 logit error; can flip near-tied expert pairs).
    """
    nc = bacc.Bacc(None)

    inp = nc.declare_dram_parameter("inp", [T_CORE, D], f32, isOutput=False)
    W = nc.declare_dram_parameter("W", [E, D], f32, isOutput=False)
    b = nc.declare_dram_parameter("b", [E], f32, isOutput=False)
    out_idx = nc.declare_dram_parameter("out_idx", [T_CORE, TOP_K], u32, isOutput=True)
    out_score = nc.declare_dram_parameter("out_score", [T_CORE, TOP_K], f32, isOutput=True)

    def mm_cast(ap):
        return ap.bitcast(f32r) if matmul_mode == "f32r" else ap

    with tile.TileContext(nc) as tc:
        with (
            tc.tile_pool(name="const", bufs=1) as const_pool,
            tc.tile_pool(name="nat", bufs=2) as nat_pool,
            tc.tile_pool(name="inpT", bufs=2) as inpT_pool,
            tc.tile_pool(name="inpTh", bufs=2) as inpTh_pool,
            tc.tile_pool(name="inpTl", bufs=2) as inpTl_pool,
            tc.tile_pool(name="small", bufs=4) as small_pool,
            tc.tile_pool(name="acc", bufs=1) as acc_pool,
            tc.tile_pool(name="ps_t", bufs=2, space="PSUM") as ps_t_pool,
            tc.tile_pool(name="ps_g", bufs=2, space="PSUM") as ps_g_pool,
            tc.tile_pool(name="ps_s", bufs=2, space="PSUM") as ps_s_pool,
        ):
            # ---- input stream first so the big DMAs hit the queues early ----
            nats = []
            for g in range(N_GROUPS):
                nat = nat_pool.tile([128, SUBT, D], f32)
                nc.sync.dma_start(
                    out=nat[:],
                    in_=inp[g * GROUP:(g + 1) * GROUP, :].rearrange(
                        "(s p) d -> p s d", p=128
                    ),
                )
                nats.append(nat)

            # ---- constants / prologue ----
            ident = const_pool.tile([128, 128], f32)
            make_identity(nc, ident)

            # bias replicated to all 128 partitions during DMA
            bias_sb = const_pool.tile([128, E], f32)
            b_ap = b[:]
            bias_bcast = bass.AP(
                tensor=b_ap.tensor, offset=b_ap.offset, ap=[[0, 128]] + list(b_ap.ap)
            )
            nc.gpsimd.dma_start(out=bias_sb[:], in_=bias_bcast)

            # W [64, 2048] -> WT_sb [128, kch, 64] (d-major). In f32r mode the
            # drain copies write through an f32r-bitcast AP: walrus requires
            # f32r matmul inputs to be rounded by the producer.
            w_sb = const_pool.tile([E, D], f32)
            nc.scalar.dma_start(out=w_sb[:], in_=W[:, :])
            WT_sb = const_pool.tile([128, KCH, E], f32)
            WTh = const_pool.tile([128, KCH, E], f16)
            WTl = const_pool.tile([128, KCH, E], f16)
            for k in range(KCH):
                pw = ps_s_pool.tile([128, E], f32, tag="ps_small")
                nc.tensor.transpose(pw[:], w_sb[:, k * 128:(k + 1) * 128], ident[:E, :E])
                nc.vector.tensor_copy(mm_cast(WT_sb[:, k, :]), pw[:])
                if matmul_mode == "fp16x3":
                    nc.vector.tensor_copy(WTh[:, k, :], pw[:])
                    nc.vector.tensor_sub(WTl[:, k, :], pw[:], WTh[:, k, :])

            # per-core collected top-8 values / indices
            vals8 = acc_pool.tile([128, N_TILES, 8], f32)
            idx8 = acc_pool.tile([128, N_TILES, 8], u32)

            pending = None  # deferred gate epilogue of the previous group

            def gate_epilogue(g, gate_ps):
                # gate.T [64, 512] psum -> sbuf, then 4 PE transposes back to
                # [128 tok, 64] and per-tile top-k.
                gsbT = small_pool.tile([E, GROUP], f32, tag="gsbT")
                nc.vector.tensor_copy(gsbT[:], gate_ps[:])
                for s in range(SUBT):
                    ti = g * SUBT + s
                    pt = ps_s_pool.tile([128, E], f32, tag="ps_small")
                    nc.tensor.transpose(
                        pt[:], gsbT[:, s * 128:(s + 1) * 128], ident[:E, :E]
                    )
                    gate_sb = small_pool.tile([128, E], f32, tag="gate_sb")
                    nc.vector.tensor_add(gate_sb[:], pt[:], bias_sb[:])
                    nc.vector.max(out=vals8[:, ti], in_=gate_sb[:])
                    nc.vector.max_index(
                        out=idx8[:, ti], in_max=vals8[:, ti], in_values=gate_sb[:]
                    )

            # ---- main loop over 512-token groups ----
            for g in range(N_GROUPS):
                nat = nats[g]
                if matmul_mode == "fp16x3":
                    inpTh = inpTh_pool.tile([128, KCH, GROUP], f16)
                    inpTl = inpTl_pool.tile([128, KCH, GROUP], f16)
                else:
                    inpT = inpT_pool.tile([128, KCH, GROUP], f32)
                # transpose 2 k-chunks into one 2-bank psum tile, then one
                # bigger drain; drains alternate DVE / ACT (2:1)
                for kk in range(KCH // 2):
                    pt = ps_t_pool.tile([128, 2, GROUP], f32)
                    for half in range(2):
                        k = 2 * kk + half
                        for s in range(SUBT):
                            nc.tensor.matmul(
                                pt[:, half, s * 128:(s + 1) * 128],
                                nat[:, s, k * 128:(k + 1) * 128],
                                ident[:],
                                is_transpose=True,
                                start=(s == 0),
                                stop=(s == SUBT - 1),
                            )
                    if matmul_mode == "fp16x3":
                        dh = inpTh[:, 2 * kk:2 * kk + 2, :]
                        dl = inpTl[:, 2 * kk:2 * kk + 2, :]
                        nc.vector.tensor_copy(dh, pt[:])
                        nc.vector.tensor_sub(dl, pt[:], dh)
                    else:
                        dst = inpT[:, 2 * kk:2 * kk + 2, :]
                        nc.vector.tensor_copy(mm_cast(dst), pt[:])

                if pending is not None:
                    gate_epilogue(*pending)
                    pending = None

                gate_ps = ps_g_pool.tile([E, GROUP], f32, tag="gate_ps")
                if matmul_mode == "fp16x3":
                    passes = [(WTh, inpTh), (WTh, inpTl), (WTl, inpTh)]
                    n_mm = KCH * len(passes)
                    i_mm = 0
                    for k in range(KCH):
                        for (wt, xt) in passes:
                            nc.tensor.matmul(
                                gate_ps[:],
                                wt[:, k, :],
                                xt[:, k, :],
                                start=(i_mm == 0),
                                stop=(i_mm == n_mm - 1),
                            )
                            i_mm += 1
                elif matmul_mode == "f32row":
                    # concurrent K=64 row-packed pairs, one accumulation group
                    for k in range(KCH):
                        for h in range(2):
                            nc.tensor.matmul(
                                gate_ps[:],
                                WT_sb[h * E:(h + 1) * E, k, :],
                                inpT[h * E:(h + 1) * E, k, :],
                                start=(k == 0 and h == 0),
                                stop=(k == KCH - 1 and h == 1),
                                skip_group_check=True,
                            )
                else:
                    for k in range(KCH):
                        nc.tensor.matmul(
                            gate_ps[:],
                            mm_cast(WT_sb[:, k, :]),
                            mm_cast(inpT[:, k, :]),
                            start=(k == 0),
                            stop=(k == KCH - 1),
                        )
                pending = (g, gate_ps)

            gate_epilogue(*pending)

            # ---- batched softmax over the two top logits ----
            v0 = vals8[:, :, 0]
            v1 = vals8[:, :, 1]
            d_t = small_pool.tile([128, N_TILES], f32, tag="soft")
            e_t = small_pool.tile([128, N_TILES], f32, tag="soft")
            den_t = small_pool.tile([128, N_TILES], f32, tag="soft")
            score_sb = acc_pool.tile([128, N_TILES, TOP_K], f32)
            nc.vector.tensor_sub(d_t[:], v1, v0)
            nc.scalar.activation(e_t[:], d_t[:], mybir.ActivationFunctionType.Exp)
            nc.vector.tensor_scalar_add(den_t[:], e_t[:], 1.0)
            nc.vector.reciprocal(score_sb[:, :, 0], den_t[:])
            nc.vector.tensor_mul(score_sb[:, :, 1], e_t[:], score_sb[:, :, 0])

            # ---- outputs ----
            nc.sync.dma_start(
                out=out_idx.rearrange("(tl p) k -> p tl k", p=128),
                in_=idx8[:, :, 0:TOP_K],
            )
            nc.sync.dma_start(
                out=out_score.rearrange("(tl p) k -> p tl k", p=128),
                in_=score_sb[:],
            )

    nc.finalize()
    return nc


def build_gate_kernel_dma(n_kb: int = 4) -> bass.Bass:
    """v5: host supplies inp pre-split into fp16 hi/lo (Xh, Xl). The d-major
    layout is produced by the DMA xbar hardware transpose (2-byte dtypes
    only — this is what the fp16 split unlocks), so there are no PE
    transposes and no PSUM drains at all. Gate logits are computed as
    three fp16 matmul passes (XhWh + XlWh + XhWl, error ~1e-7 of fp32).
    """
    nc = bacc.Bacc(None)

    KB = KCH // n_kb  # k-chunks per transposed DMA

    Xh = nc.declare_dram_parameter("Xh", [T_CORE, D], f16, isOutput=False)
    Xl = nc.declare_dram_parameter("Xl", [T_CORE, D], f16, isOutput=False)
    W = nc.declare_dram_parameter("W", [E, D], f32, isOutput=False)
    b = nc.declare_dram_parameter("b", [E], f32, isOutput=False)
    out_idx = nc.declare_dram_parameter("out_idx", [T_CORE, TOP_K], u32, isOutput=True)
    out_score = nc.declare_dram_parameter("out_score", [T_CORE, TOP_K], f32, isOutput=True)

    with tile.TileContext(nc) as tc:
        with (
            tc.tile_pool(name="const", bufs=1) as const_pool,
            tc.tile_pool(name="xt", bufs=1) as xt_pool,
            tc.tile_pool(name="small", bufs=4) as small_pool,
            tc.tile_pool(name="acc", bufs=1) as acc_pool,
            tc.tile_pool(name="ps_g", bufs=1, space="PSUM") as ps_g_pool,
            tc.tile_pool(name="ps_s", bufs=2, space="PSUM") as ps_s_pool,
        ):
            # ---- transposed input stream: out[p, j, t] = X[t, kb*KB*128 + 128j + p]
            XhT, XlT = [], []
            for kb in range(n_kb):
                cols = slice(kb * KB * 128, (kb + 1) * KB * 128)
                th = xt_pool.tile([128, KB, T_CORE], f16, tag=f"xh{kb}", name=f"xh{kb}")
                nc.sync.dma_start(out=th[:], in_=Xh[:, cols], transpose=True)
                tl = xt_pool.tile([128, KB, T_CORE], f16, tag=f"xl{kb}", name=f"xl{kb}")
                nc.sync.dma_start(out=tl[:], in_=Xl[:, cols], transpose=True)
                XhT.append(th)
                XlT.append(tl)

            # ---- constants / prologue ----
            ident = const_pool.tile([128, 128], f32)
            make_identity(nc, ident)

            bias_sb = const_pool.tile([128, E], f32)
            b_ap = b[:]
            bias_bcast = bass.AP(
                tensor=b_ap.tensor, offset=b_ap.offset, ap=[[0, 128]] + list(b_ap.ap)
            )
            nc.gpsimd.dma_start(out=bias_sb[:], in_=bias_bcast)

            # W [64, 2048] -> d-major fp16 hi/lo splits
            w_sb = const_pool.tile([E, D], f32)
            nc.scalar.dma_start(out=w_sb[:], in_=W[:, :])
            WTh = const_pool.tile([128, KCH, E], f16)
            WTl = const_pool.tile([128, KCH, E], f16)
            for k in range(KCH):
                pw = ps_s_pool.tile([128, E], f32, tag="ps_small")
                nc.tensor.transpose(pw[:], w_sb[:, k * 128:(k + 1) * 128], ident[:E, :E])
                nc.vector.tensor_copy(WTh[:, k, :], pw[:])
                nc.vector.tensor_sub(WTl[:, k, :], pw[:], WTh[:, k, :])

            vals8 = acc_pool.tile([128, N_TILES, 8], f32)
            idx8 = acc_pool.tile([128, N_TILES, 8], u32)

            def gate_epilogue(g, gate_ps):
                gsbT = small_pool.tile([E, GROUP], f32, tag="gsbT")
                nc.vector.tensor_copy(gsbT[:], gate_ps[:])
                for s in range(SUBT):
                    ti = g * SUBT + s
                    pt = ps_s_pool.tile([128, E], f32, tag="ps_small")
                    nc.tensor.transpose(
                        pt[:], gsbT[:, s * 128:(s + 1) * 128], ident[:E, :E]
                    )
                    gate_sb = small_pool.tile([128, E], f32, tag="gate_sb")
                    nc.vector.tensor_add(gate_sb[:], pt[:], bias_sb[:])
                    nc.vector.max(out=vals8[:, ti], in_=gate_sb[:])
                    nc.vector.max_index(
                        out=idx8[:, ti], in_max=vals8[:, ti], in_values=gate_sb[:]
                    )

            # ---- gate matmuls: kb-outer so PE has 4 token-groups of work per
            # arriving DMA; 4 PSUM accumulation groups stay open throughout
            gate_pss = [
                ps_g_pool.tile([E, GROUP], f32, tag=f"gate_ps{g}", name=f"gate_ps{g}")
                for g in range(N_GROUPS)
            ]
            NPASS = 3
            for kb in range(n_kb):
                for g in range(N_GROUPS):
                    tok = slice(g * GROUP, (g + 1) * GROUP)
                    for j in range(KB):
                        k = kb * KB + j
                        for p_i, (wt, xt) in enumerate(
                            [(WTh, XhT[kb]), (WTh, XlT[kb]), (WTl, XhT[kb])]
                        ):
                            nc.tensor.matmul(
                                gate_pss[g][:],
                                wt[:, k, :],
                                xt[:, j, tok],
                                start=(kb == 0 and j == 0 and p_i == 0),
                                stop=(kb == n_kb - 1 and j == KB - 1 and p_i == NPASS - 1),
                            )
                    if kb == n_kb - 1:
                        gate_epilogue(g, gate_pss[g])

            # ---- batched softmax over the two top logits ----
            v0 = vals8[:, :, 0]
            v1 = vals8[:, :, 1]
            d_t = small_pool.tile([128, N_TILES], f32, tag="soft")
            e_t = small_pool.tile([128, N_TILES], f32, tag="soft")
            den_t = small_pool.tile([128, N_TILES], f32, tag="soft")
            score_sb = acc_pool.tile([128, N_TILES, TOP_K], f32)
            nc.vector.tensor_sub(d_t[:], v1, v0)
            nc.scalar.activation(e_t[:], d_t[:], mybir.ActivationFunctionType.Exp)
            nc.vector.tensor_scalar_add(den_t[:], e_t[:], 1.0)
            nc.vector.reciprocal(score_sb[:, :, 0], den_t[:])
            nc.vector.tensor_mul(score_sb[:, :, 1], e_t[:], score_sb[:, :, 0])

            # ---- outputs ----
            nc.sync.dma_start(
                out=out_idx.rearrange("(tl p) k -> p tl k", p=128),
                in_=idx8[:, :, 0:TOP_K],
            )
            nc.sync.dma_start(
                out=out_score.rearrange("(tl p) k -> p tl k", p=128),
                in_=score_sb[:],
            )

    nc.finalize()
    return nc


_NC_CACHE: dict = {}


def _get_nc(matmul_mode: str) -> bass.Bass:
    if matmul_mode not in _NC_CACHE:
        if matmul_mode == "fp16dma":
            _NC_CACHE[matmul_mode] = build_gate_kernel_dma()
        else:
            _NC_CACHE[matmul_mode] = build_gate_kernel(matmul_mode)
    return _NC_CACHE[matmul_mode]


def kernel(inp, W, b, matmul_mode: str = "fp16dma", trace: bool = False, **run_kwargs):
    from concourse.bass_utils import run_bass_kernel_spmd

    inp = np.ascontiguousarray(np.asarray(inp, dtype=np.float32))
    W = np.ascontiguousarray(np.asarray(W, dtype=np.float32))
    b = np.ascontiguousarray(np.asarray(b, dtype=np.float32))
    assert inp.shape == (TOKENS, D) and W.shape == (E, D) and b.shape == (E,)

    nc = _get_nc(matmul_mode)
    if matmul_mode == "fp16dma":
        # fp16 hi/lo marshalling split (exact: inp == Xh + Xl to ~2^-22 rel)
        Xh = inp.astype(np.float16)
        Xl = (inp - Xh.astype(np.float32)).astype(np.float16)
        in_maps = [
            {
                "Xh": Xh[c * T_CORE:(c + 1) * T_CORE],
                "Xl": Xl[c * T_CORE:(c + 1) * T_CORE],
                "W": W,
                "b": b,
            }
            for c in range(N_CORES)
        ]
    else:
        in_maps = [
            {"inp": inp[c * T_CORE:(c + 1) * T_CORE], "W": W, "b": b}
            for c in range(N_CORES)
        ]
    res = run_bass_kernel_spmd(
        nc, in_maps, core_ids=list(range(N_CORES)), trace=trace, **run_kwargs
    )
    kernel.last_result = res

    idx = np.concatenate([res.results[c]["out_idx"] for c in range(N_CORES)], axis=0)
    score = np.concatenate(
        [res.results[c]["out_score"] for c in range(N_CORES)], axis=0
    )
    gate_top_k_idx = idx.astype(np.int32).reshape(-1)
    gate_score = score.reshape(TOKENS, 1, TOP_K).astype(np.float32)
    return (gate_top_k_idx, gate_score)
